# revision 13
# baseline (speedup 1.0000x reference)
"""3-layer GCN (ContrastiveGNN) on 8 Trainium2 NeuronCores.

Strategy (dst-sharded edge partition, "1D graph partition"):
  - Nodes are split into 8 blocks of 12500 dsts; device i owns block i and all
    edges whose dst lands in its block (plus that block's self-loops).
  - Math reorder: for each GCN layer,
        out = D^-1/2 (A+I) D^-1/2 (h W) + b  ==  dis_d * (sum_{e->d} T[src]) @ W + b
    with T = dis * h (row-scaled activations).  Aggregation happens BEFORE the
    dense transform, so the gather tables carry 128 features for every layer.
  - Aggregation on the tensor engine: edges are sorted by (window-batch,
    src-block-pair, dst-window); each 128-edge group contributes
    one_hot[e, dst_local].T @ gathered[e, feat] accumulated in PSUM per
    128-dst window.  One-hot matrices are built on DVE via iota compare.
  - Gathers use the SWDGE dma_gather custom instruction (int16 indices =>
    the 100352-row table is addressed in 4 block-pair regions of 25088 rows).
  - Tables are bf16 (PSUM accumulation f32); between layers the 8 per-device
    table blocks are exchanged with an AllGather collective.
  - All 8 devices run one SPMD program: per-(batch,pair,window) group counts
    are padded to the max over devices, so instruction streams are identical
    and only the input data (indices, one-hot selectors, dis) differs.
"""

import numpy as np
import ml_dtypes

BF16 = ml_dtypes.bfloat16

N = 100000
F = 128
DOUT = 64
M = 8
BLK = N // M            # 12500 dst nodes per device
P = 128
WPD = (BLK + P - 1) // P  # 98 windows per device
BLKP = WPD * P            # 12544 padded block rows
TROWS = M * BLKP          # 100352 table rows
PAIR = 2 * BLKP           # 25088 rows per src-block-pair region (int16-addressable)
NPAIR = 4
WB = 6                    # windows per gather batch
NBATCH = (WPD + WB - 1) // WB
OB = 8                    # one-hot groups built per DVE op
PADREL = BLKP - 1         # pair-local row of a guaranteed-zero table row


class _Call:
    __slots__ = ("ic0", "c16", "dc0", "c128", "nslots", "wins", "winmap")


def _preprocess(x, edge_index, W1, b1, W2, b2, W3, b3):
    """Host-side index plumbing + input staging. Returns (meta, per-core in_maps,
    reusable static arrays)."""
    x = np.asarray(x, np.float32)
    ei = np.asarray(edge_index)
    src = ei[0].astype(np.int64)
    dst = ei[1].astype(np.int64)
    loop = np.arange(N, dtype=np.int64)
    s_all = np.concatenate([src, loop])
    d_all = np.concatenate([dst, loop])

    deg = np.bincount(d_all, minlength=N).astype(np.float32)
    dis = (1.0 / np.sqrt(deg)).astype(np.float32)

    # layer-1 gather table: dis-scaled input features, block layout with
    # zeroed pad rows (rows BLK..BLKP-1 of each block)
    xs = x * dis[:, None]
    T1 = np.zeros((TROWS, F), BF16)
    for j in range(M):
        T1[j * BLKP : j * BLKP + BLK] = xs[j * BLK : (j + 1) * BLK].astype(BF16)

    dev = d_all // BLK
    j_src = s_all // BLK
    trow = j_src * BLKP + (s_all - j_src * BLK)
    p_pair = j_src // 2
    rel = (trow - p_pair * PAIR).astype(np.int64)  # 0..PAIR-1
    dloc = d_all - dev * BLK
    w_arr = dloc // P
    dwin = dloc - w_arr * P
    wb_arr = w_arr // WB
    bkey = (wb_arr * NPAIR + p_pair) * WPD + w_arr
    NBUCK = NBATCH * NPAIR * WPD

    cnt = np.zeros((M, NBUCK), np.int64)
    for i in range(M):
        cnt[i] = np.bincount(bkey[dev == i], minlength=NBUCK)
    cmax = cnt.max(axis=0)

    # common (SPMD-uniform) padded group counts; >=1 group per valid bucket
    meta_calls = {}
    gtot = np.zeros(WPD, np.int64)
    bucket_order = []   # (bucket_id, slot_offset, padded_slots)
    ic, dc, off = 0, 0, 0
    for wb in range(NBATCH):
        w0 = wb * WB
        wcnt = min(WB, WPD - w0)
        for p in range(NPAIR):
            c = _Call()
            c.ic0, c.dc0 = ic, dc
            c.wins = []
            call_slots = 0
            for w in range(w0, w0 + wcnt):
                bid = (wb * NPAIR + p) * WPD + w
                G = max(1, -(-int(cmax[bid]) // P))
                c.wins.append((w, call_slots // P, call_slots // P + G))
                bucket_order.append((bid, off + call_slots, G * P))
                call_slots += G * P
                gtot[w] += G
            c.nslots = call_slots
            c.c16 = call_slots // 16
            c.c128 = call_slots // P
            c.winmap = {w: (glo, ghi) for (w, glo, ghi) in c.wins}
            ic += c.c16
            dc += c.c128
            off += call_slots
            meta_calls[(wb, p)] = c
    tot_slots = off

    meta = {
        "calls": meta_calls,
        "gtot": gtot,
        "sc16": tot_slots // 16,
        "sc128": tot_slots // P,
        "tot_slots": tot_slots,
    }

    # per-device padded slot arrays
    iota_np = np.tile(np.arange(P, dtype=np.float32).astype(BF16), (P, 1)).reshape(
        P, 1, P
    )
    ident_np = np.eye(P, dtype=np.float32).astype(BF16)
    w1b = np.asarray(W1, np.float32).astype(BF16)
    w2b = np.asarray(W2, np.float32).astype(BF16)
    w3b = np.asarray(W3, np.float32).astype(BF16)
    b1f = np.tile(np.asarray(b1, np.float32), (P, 1))
    b2f = np.tile(np.asarray(b2, np.float32), (P, 1))
    b3f = np.tile(np.asarray(b3, np.float32), (P, 1))

    in_maps = []
    for i in range(M):
        m = dev == i
        bk = bkey[m]
        o = np.argsort(bk, kind="stable")
        bk_s = bk[o]
        rel_s = rel[m][o].astype(np.int16)
        dw_s = dwin[m][o].astype(np.float32)

        idxfl = np.full(tot_slots, PADREL, np.int16)
        dlfl = np.full(tot_slots, -1.0, np.float32)
        bids = np.array([b[0] for b in bucket_order], np.int64)
        starts = np.searchsorted(bk_s, bids)
        for (bid, so, pslots), st in zip(bucket_order, starts):
            cband = int(cnt[i][bid])
            if cband:
                idxfl[so : so + cband] = rel_s[st : st + cband]
                dlfl[so : so + cband] = dw_s[st : st + cband]

        # wrap per call: idx -> [16, c16] tiled to 128 partitions; dl -> [128, c128]
        i16_parts, d128_parts = [], []
        for wb in range(NBATCH):
            for p in range(NPAIR):
                c = meta_calls[(wb, p)]
                so = None
        # offsets per call follow bucket_order grouping; rebuild from cumsum
        off2 = 0
        for wb in range(NBATCH):
            for p in range(NPAIR):
                c = meta_calls[(wb, p)]
                seg_i = idxfl[off2 : off2 + c.nslots]
                seg_d = dlfl[off2 : off2 + c.nslots]
                i16_parts.append(seg_i.reshape(-1, 16).T)
                d128_parts.append(seg_d.reshape(-1, P).T)
                off2 += c.nslots
        idx16 = np.tile(np.concatenate(i16_parts, axis=1), (8, 1))
        dl128 = np.concatenate(d128_parts, axis=1).astype(BF16)

        disp = np.zeros(BLKP, np.float32)
        disp[:BLK] = dis[i * BLK : (i + 1) * BLK]
        disb = disp.reshape(WPD, P).T.copy()

        in_maps.append(
            {
                "t1": T1,
                "idx16": idx16,
                "dl128": dl128,
                "disb": disb,
                "iota": iota_np,
                "ident": ident_np,
                "w1": w1b,
                "w2": w2b,
                "w3": w3b,
                "b1f": b1f,
                "b2f": b2f,
                "b3f": b3f,
            }
        )
    return meta, in_maps


def _build_program(meta):
    import os
    import concourse.bacc as bacc
    import concourse.mybir as mybir
    import concourse.tile as tile
    from contextlib import ExitStack

    dbg_layers = int(os.environ.get("GNN_LAYERS", "3"))
    dbg_bcap = int(os.environ.get("GNN_BATCH_CAP", str(NBATCH)))
    dbg_coll = os.environ.get("GNN_COLL", "1") == "1"
    nqueues = int(os.environ.get("GNN_QUEUES", "4"))
    tagg = os.environ.get("GNN_TAGG", "1") == "1"
    ohb = os.environ.get("GNN_OHB", "1") == "1"
    gchunk = int(os.environ.get("GNN_GCHUNK", "8"))

    dt = mybir.dt
    nc = bacc.Bacc(
        "TRN2",
        target_bir_lowering=False,
        debug=False,
        num_devices=M,
        num_swdge_queues=nqueues,
    )

    t1 = nc.dram_tensor("t1", [TROWS, F], dt.bfloat16, kind="ExternalInput")
    idxd = nc.dram_tensor("idx16", [P, meta["sc16"]], dt.int16, kind="ExternalInput")
    dld = nc.dram_tensor("dl128", [P, meta["sc128"]], dt.bfloat16, kind="ExternalInput")
    disd = nc.dram_tensor("disb", [P, WPD], dt.float32, kind="ExternalInput")
    iod = nc.dram_tensor("iota", [P, 1, P], dt.bfloat16, kind="ExternalInput")
    idnd = nc.dram_tensor("ident", [P, P], dt.bfloat16, kind="ExternalInput")
    w1d = nc.dram_tensor("w1", [F, F], dt.bfloat16, kind="ExternalInput")
    w2d = nc.dram_tensor("w2", [F, F], dt.bfloat16, kind="ExternalInput")
    w3d = nc.dram_tensor("w3", [F, DOUT], dt.bfloat16, kind="ExternalInput")
    b1d = nc.dram_tensor("b1f", [P, F], dt.float32, kind="ExternalInput")
    b2d = nc.dram_tensor("b2f", [P, F], dt.float32, kind="ExternalInput")
    b3d = nc.dram_tensor("b3f", [P, DOUT], dt.float32, kind="ExternalInput")
    outd = nc.dram_tensor("out", [BLKP, DOUT], dt.float32, kind="ExternalOutput")

    with tile.TileContext(nc) as tc, ExitStack() as ctx:
        const = ctx.enter_context(tc.tile_pool(name="const", bufs=1))
        dram = ctx.enter_context(tc.tile_pool(name="dram", bufs=1, space="DRAM"))
        ipool = ctx.enter_context(tc.tile_pool(name="ip", bufs=6))
        dpool = ctx.enter_context(tc.tile_pool(name="dp", bufs=6))
        gpool = ctx.enter_context(tc.tile_pool(name="gp", bufs=6))
        ohpool = ctx.enter_context(tc.tile_pool(name="ohp", bufs=6))
        lhpool = ctx.enter_context(tc.tile_pool(name="lhp", bufs=3))
        zbpool = ctx.enter_context(tc.tile_pool(name="zbp", bufs=4))
        stage = ctx.enter_context(tc.tile_pool(name="stage", bufs=2))
        apsum = ctx.enter_context(tc.tile_pool(name="apsum", bufs=4, space="PSUM"))
        zpsum = ctx.enter_context(tc.tile_pool(name="zpsum", bufs=2, space="PSUM"))
        if not tagg:
            upool = ctx.enter_context(tc.tile_pool(name="up", bufs=3))
            tpsum = ctx.enter_context(tc.tile_pool(name="tpsum", bufs=2, space="PSUM"))

        def cload(name, dram_t, shape, dtype):
            tl = const.tile(shape, dtype, name=name)
            nc.sync.dma_start(out=tl[:], in_=dram_t[:])
            return tl

        iot = cload("iot", iod, [P, 1, P], dt.bfloat16)
        idn = cload("idn", idnd, [P, P], dt.bfloat16)
        dis_t = cload("dis_t", disd, [P, WPD], dt.float32)
        w1t = cload("w1t", w1d, [F, F], dt.bfloat16)
        w2t = cload("w2t", w2d, [F, F], dt.bfloat16)
        w3t = cload("w3t", w3d, [F, DOUT], dt.bfloat16)
        b1t = cload("b1t", b1d, [P, F], dt.float32)
        b2t = cload("b2t", b2d, [P, F], dt.float32)
        b3t = cload("b3t", b3d, [P, DOUT], dt.float32)

        tin2 = dram.tile([BLKP, F], dt.bfloat16, name="tin2")
        tin3 = dram.tile([BLKP, F], dt.bfloat16, name="tin3")
        tf2 = dram.tile([TROWS, F], dt.bfloat16, addr_space="Shared", name="tf2")
        tf3 = dram.tile([TROWS, F], dt.bfloat16, addr_space="Shared", name="tf3")

        calls = meta["calls"]
        gtot = meta["gtot"]

        qctr = [0]

        def do_layer(l, src_of, wt, bt, tst, tin=None, tfull=None):
            gctr = [0] * WPD
            for wb in range(min(NBATCH, dbg_bcap)):
                w0 = wb * WB
                wcnt = min(WB, WPD - w0)
                gts, ohs = [], []
                for p in range(NPAIR):
                    c = calls[(wb, p)]
                    it = ipool.tile([P, c.c16], dt.int16, tag="idx", name=f"it{l}_{wb}_{p}")
                    nc.sync.dma_start(out=it[:], in_=idxd[:, c.ic0 : c.ic0 + c.c16])
                    dt_ = dpool.tile(
                        [P, c.c128, 1], dt.bfloat16, tag="dl", name=f"dl{l}_{wb}_{p}"
                    )
                    nc.sync.dma_start(
                        out=dt_[:],
                        in_=dld[:, c.dc0 : c.dc0 + c.c128].rearrange(
                            "p (c o) -> p c o", o=1
                        ),
                    )
                    gt = gpool.tile(
                        [P, c.c128, F], dt.bfloat16, tag="g", name=f"gt{l}_{wb}_{p}"
                    )
                    for k0 in range(0, c.c128, gchunk):
                        kc = min(gchunk, c.c128 - k0)
                        nc.gpsimd.dma_gather(
                            gt[:, k0 : k0 + kc, :],
                            src_of(p),
                            it[:, k0 * 8 : (k0 + kc) * 8],
                            kc * P,
                            kc * P,
                            F,
                            queue_num=qctr[0] % nqueues,
                        )
                        qctr[0] += 1
                    oh = ohpool.tile(
                        [P, c.c128, P], dt.bfloat16, tag="oh", name=f"oh{l}_{wb}_{p}"
                    )
                    ohstep = c.c128 if ohb else OB
                    for c0 in range(0, c.c128, ohstep):
                        cb = min(ohstep, c.c128 - c0)
                        nc.vector.tensor_tensor(
                            out=oh[:, c0 : c0 + cb, :],
                            in0=dt_[:, c0 : c0 + cb, :].to_broadcast([P, cb, P]),
                            in1=iot[:].to_broadcast([P, cb, P]),
                            op=mybir.AluOpType.is_equal,
                        )
                    gts.append(gt)
                    ohs.append(oh)
                for w in range(w0, w0 + wcnt):
                    # aggregation: tagg => aggT[feat, dst] = sum gt.T @ oh,
                    # else agg[dst, feat] = sum oh.T @ gt
                    agg = apsum.tile([P, P], dt.float32, tag="agg", name=f"agg{l}_{w}")
                    for p in range(NPAIR):
                        c = calls[(wb, p)]
                        glo, ghi = c.winmap[w]
                        for g in range(glo, ghi):
                            st = gctr[w] == 0
                            gctr[w] += 1
                            sp = gctr[w] == gtot[w]
                            nc.tensor.matmul(
                                agg[:],
                                lhsT=gts[p][:, g, :] if tagg else ohs[p][:, g, :],
                                rhs=ohs[p][:, g, :] if tagg else gts[p][:, g, :],
                                start=st,
                                stop=sp,
                            )
                    if tagg:
                        lh = lhpool.tile([P, P], dt.bfloat16, tag="lh", name=f"lh{l}_{w}")
                        nc.vector.tensor_copy(out=lh[:], in_=agg[:])
                    else:
                        u = upool.tile([P, P], dt.bfloat16, tag="u", name=f"u{l}_{w}")
                        nc.vector.tensor_scalar(
                            u[:], agg[:], dis_t[:, w : w + 1], None, mybir.AluOpType.mult
                        )
                        tp = tpsum.tile([P, P], dt.bfloat16, tag="tp", name=f"tp{l}_{w}")
                        nc.tensor.transpose(tp[:], u[:], idn[:])
                        lh = lhpool.tile([P, P], dt.bfloat16, tag="lh", name=f"lh{l}_{w}")
                        nc.vector.tensor_copy(out=lh[:], in_=tp[:])
                    zw = zpsum.tile(
                        [P, F if l < 2 else DOUT], dt.float32, tag="zp", name=f"z{l}_{w}"
                    )
                    nc.tensor.matmul(zw[:], lhsT=lh[:], rhs=wt[:], start=True, stop=True)
                    if tagg:
                        zb = zbpool.tile(
                            [P, F if l < 2 else DOUT], dt.float32, tag="zb",
                            name=f"zb{l}_{w}",
                        )
                        nc.vector.tensor_scalar(
                            zb[:], zw[:], dis_t[:, w : w + 1], None, mybir.AluOpType.mult
                        )
                    else:
                        zb = zw
                    if l < 2:
                        zc = zbpool.tile([P, F], dt.float32, tag="zb", name=f"zc{l}_{w}")
                        nc.vector.tensor_tensor(
                            out=zc[:], in0=zb[:], in1=bt[:], op=mybir.AluOpType.add
                        )
                        nc.scalar.activation(
                            tst[:, w * F : (w + 1) * F],
                            zc[:],
                            mybir.ActivationFunctionType.Relu,
                            scale=dis_t[:, w : w + 1],
                        )
                    else:
                        nc.vector.tensor_tensor(
                            out=tst[:, w * DOUT : (w + 1) * DOUT],
                            in0=zb[:],
                            in1=bt[:],
                            op=mybir.AluOpType.add,
                        )
            if l < 2:
                nc.sync.dma_start(
                    out=tin[:].rearrange("(w p) f -> p w f", p=P),
                    in_=tst[:].rearrange("p (w f) -> p w f", f=F),
                )
                if dbg_coll:
                    nc.gpsimd.collective_compute(
                        "AllGather",
                        mybir.AluOpType.bypass,
                        replica_groups=[list(range(M))],
                        ins=[tin.opt()],
                        outs=[tfull.opt()],
                    )
                else:
                    nc.sync.dma_start(
                        out=tfull[0:BLKP, :].rearrange("(w p) f -> p w f", p=P),
                        in_=tst[:].rearrange("p (w f) -> p w f", f=F),
                    )
            else:
                nc.sync.dma_start(
                    out=outd[:].rearrange("(w p) f -> p w f", p=P),
                    in_=tst[:].rearrange("p (w f) -> p w f", f=DOUT),
                )

        ts1 = stage.tile([P, WPD * F], dt.bfloat16, tag="tstage", name="ts1")
        do_layer(0, lambda p: t1[p * PAIR : (p + 1) * PAIR, :], w1t, b1t, ts1, tin2, tf2)
        if dbg_layers >= 2:
            ts2 = stage.tile([P, WPD * F], dt.bfloat16, tag="tstage", name="ts2")
            do_layer(
                1, lambda p: tf2[p * PAIR : (p + 1) * PAIR, :], w2t, b2t, ts2, tin3, tf3
            )
        if dbg_layers >= 3:
            ts3 = stage.tile([P, WPD * DOUT], dt.float32, tag="tstage", name="ts3")
            do_layer(2, lambda p: tf3[p * PAIR : (p + 1) * PAIR, :], w3t, b3t, ts3)
        else:
            zts = stage.tile([P, WPD * DOUT], dt.float32, tag="tstage", name="zts")
            nc.vector.memset(zts[:], 0.0)
            nc.sync.dma_start(
                out=outd[:].rearrange("(w p) f -> p w f", p=P),
                in_=zts[:].rearrange("p (w f) -> p w f", f=DOUT),
            )

    nc.compile()
    return nc


_CACHE = {}


def _get_program(meta):
    import os

    key = (
        meta["sc16"],
        meta["sc128"],
        os.environ.get("GNN_LAYERS"),
        os.environ.get("GNN_BATCH_CAP"),
        os.environ.get("GNN_COLL"),
        os.environ.get("GNN_QUEUES"),
        os.environ.get("GNN_TAGG"),
        os.environ.get("GNN_OHB"),
        os.environ.get("GNN_GCHUNK"),
    )
    if key not in _CACHE:
        _CACHE[key] = _build_program(meta)
    return _CACHE[key]


def run(trace=False, **inputs):
    from concourse.bass_utils import run_bass_kernel_spmd

    meta, in_maps = _preprocess(**inputs)
    nc = _get_program(meta)
    res = run_bass_kernel_spmd(nc, in_maps, core_ids=list(range(M)), trace=trace)
    out = np.empty((N, DOUT), np.float32)
    for i in range(M):
        out[i * BLK : (i + 1) * BLK] = res.results[i]["out"][:BLK]
    return out, res


def kernel(**inputs):
    out, _ = run(trace=False, **inputs)
    return out



# revision 20
# speedup vs baseline: 1.0073x; 1.0073x over previous
"""3-layer GCN (ContrastiveGNN) on 8 Trainium2 NeuronCores.

Strategy (dst-sharded edge partition, "1D graph partition"):
  - Nodes are split into 8 blocks of 12500 dsts; device i owns block i and all
    edges whose dst lands in its block (plus that block's self-loops).
  - Math reorder: for each GCN layer,
        out = D^-1/2 (A+I) D^-1/2 (h W) + b  ==  dis_d * (sum_{e->d} T[src]) @ W + b
    with T = dis * h (row-scaled activations).  Aggregation happens BEFORE the
    dense transform, so the gather tables carry 128 features for every layer.
  - Aggregation on the tensor engine: edges are sorted by (window-batch,
    src-block-pair, dst-window); each 128-edge group contributes
    one_hot[e, dst_local].T @ gathered[e, feat] accumulated in PSUM per
    128-dst window.  One-hot matrices are built on DVE via iota compare.
  - Gathers use the SWDGE dma_gather custom instruction (int16 indices =>
    the 100352-row table is addressed in 4 block-pair regions of 25088 rows).
  - Tables are bf16 (PSUM accumulation f32); between layers the 8 per-device
    table blocks are exchanged with an AllGather collective.
  - All 8 devices run one SPMD program: per-(batch,pair,window) group counts
    are padded to the max over devices, so instruction streams are identical
    and only the input data (indices, one-hot selectors, dis) differs.
"""

import numpy as np
import ml_dtypes

BF16 = ml_dtypes.bfloat16

N = 100000
F = 128
DOUT = 64
M = 8
BLK = N // M            # 12500 dst nodes per device
P = 128
WPD = (BLK + P - 1) // P  # 98 windows per device
BLKP = WPD * P            # 12544 padded block rows
TROWS = M * BLKP          # 100352 table rows
PAIR = 2 * BLKP           # 25088 rows per src-block-pair region (int16-addressable)
NPAIR = 4
WB = 6                    # windows per gather batch
NBATCH = (WPD + WB - 1) // WB
OB = 8                    # one-hot groups built per DVE op
PADREL = BLKP - 1         # pair-local row of a guaranteed-zero table row


class _Call:
    __slots__ = ("ic0", "c16", "dc0", "c128", "nslots", "wins", "winmap")


def _preprocess(x, edge_index, W1, b1, W2, b2, W3, b3):
    """Host-side index plumbing + input staging. Returns (meta, per-core in_maps,
    reusable static arrays)."""
    x = np.asarray(x, np.float32)
    ei = np.asarray(edge_index)
    src = ei[0].astype(np.int64)
    dst = ei[1].astype(np.int64)
    loop = np.arange(N, dtype=np.int64)
    s_all = np.concatenate([src, loop])
    d_all = np.concatenate([dst, loop])

    deg = np.bincount(d_all, minlength=N).astype(np.float32)
    dis = (1.0 / np.sqrt(deg)).astype(np.float32)

    # layer-1 gather table: dis-scaled input features, block layout with
    # zeroed pad rows (rows BLK..BLKP-1 of each block)
    xs = x * dis[:, None]
    T1 = np.zeros((TROWS, F), BF16)
    for j in range(M):
        T1[j * BLKP : j * BLKP + BLK] = xs[j * BLK : (j + 1) * BLK].astype(BF16)

    dev = d_all // BLK
    j_src = s_all // BLK
    trow = j_src * BLKP + (s_all - j_src * BLK)
    p_pair = j_src // 2
    rel = (trow - p_pair * PAIR).astype(np.int64)  # 0..PAIR-1
    dloc = d_all - dev * BLK
    w_arr = dloc // P
    dwin = dloc - w_arr * P
    wb_arr = w_arr // WB
    bkey = (wb_arr * NPAIR + p_pair) * WPD + w_arr
    NBUCK = NBATCH * NPAIR * WPD

    cnt = np.zeros((M, NBUCK), np.int64)
    for i in range(M):
        cnt[i] = np.bincount(bkey[dev == i], minlength=NBUCK)
    cmax = cnt.max(axis=0)

    # common (SPMD-uniform) padded group counts; >=1 group per valid bucket
    meta_calls = {}
    gtot = np.zeros(WPD, np.int64)
    bucket_order = []   # (bucket_id, slot_offset, padded_slots)
    ic, dc, off = 0, 0, 0
    for wb in range(NBATCH):
        w0 = wb * WB
        wcnt = min(WB, WPD - w0)
        for p in range(NPAIR):
            c = _Call()
            c.ic0, c.dc0 = ic, dc
            c.wins = []
            call_slots = 0
            for w in range(w0, w0 + wcnt):
                bid = (wb * NPAIR + p) * WPD + w
                G = max(1, -(-int(cmax[bid]) // P))
                c.wins.append((w, call_slots // P, call_slots // P + G))
                bucket_order.append((bid, off + call_slots, G * P))
                call_slots += G * P
                gtot[w] += G
            c.nslots = call_slots
            c.c16 = call_slots // 16
            c.c128 = call_slots // P
            c.winmap = {w: (glo, ghi) for (w, glo, ghi) in c.wins}
            ic += c.c16
            dc += c.c128
            off += call_slots
            meta_calls[(wb, p)] = c
    tot_slots = off

    meta = {
        "calls": meta_calls,
        "gtot": gtot,
        "sc16": tot_slots // 16,
        "sc128": tot_slots // P,
        "tot_slots": tot_slots,
    }

    # per-device padded slot arrays
    iota_np = np.tile(np.arange(P, dtype=np.float32).astype(BF16), (P, 1)).reshape(
        P, 1, P
    )
    ident_np = np.eye(P, dtype=np.float32).astype(BF16)
    w1b = np.asarray(W1, np.float32).astype(BF16)
    w2b = np.asarray(W2, np.float32).astype(BF16)
    w3b = np.asarray(W3, np.float32).astype(BF16)
    b1f = np.tile(np.asarray(b1, np.float32), (P, 1))
    b2f = np.tile(np.asarray(b2, np.float32), (P, 1))
    b3f = np.tile(np.asarray(b3, np.float32), (P, 1))

    in_maps = []
    for i in range(M):
        m = dev == i
        bk = bkey[m]
        o = np.argsort(bk, kind="stable")
        bk_s = bk[o]
        rel_s = rel[m][o].astype(np.int16)
        dw_s = dwin[m][o].astype(np.float32)

        idxfl = np.full(tot_slots, PADREL, np.int16)
        dlfl = np.full(tot_slots, -1.0, np.float32)
        bids = np.array([b[0] for b in bucket_order], np.int64)
        starts = np.searchsorted(bk_s, bids)
        for (bid, so, pslots), st in zip(bucket_order, starts):
            cband = int(cnt[i][bid])
            if cband:
                idxfl[so : so + cband] = rel_s[st : st + cband]
                dlfl[so : so + cband] = dw_s[st : st + cband]

        # wrap per call: idx -> [16, c16] tiled to 128 partitions; dl -> [128, c128]
        i16_parts, d128_parts = [], []
        for wb in range(NBATCH):
            for p in range(NPAIR):
                c = meta_calls[(wb, p)]
                so = None
        # offsets per call follow bucket_order grouping; rebuild from cumsum
        off2 = 0
        for wb in range(NBATCH):
            for p in range(NPAIR):
                c = meta_calls[(wb, p)]
                seg_i = idxfl[off2 : off2 + c.nslots]
                seg_d = dlfl[off2 : off2 + c.nslots]
                i16_parts.append(seg_i.reshape(-1, 16).T)
                d128_parts.append(seg_d.reshape(-1, P).T)
                off2 += c.nslots
        idx16 = np.tile(np.concatenate(i16_parts, axis=1), (8, 1))
        dl128 = np.concatenate(d128_parts, axis=1).astype(BF16)

        disp = np.zeros(BLKP, np.float32)
        disp[:BLK] = dis[i * BLK : (i + 1) * BLK]
        disb = disp.reshape(WPD, P).T.copy()

        in_maps.append(
            {
                "t1": T1,
                "idx16": idx16,
                "dl128": dl128,
                "disb": disb,
                "iota": iota_np,
                "ident": ident_np,
                "w1": w1b,
                "w2": w2b,
                "w3": w3b,
                "b1f": b1f,
                "b2f": b2f,
                "b3f": b3f,
            }
        )
    return meta, in_maps


def _build_program(meta):
    import os
    import concourse.bacc as bacc
    import concourse.mybir as mybir
    import concourse.tile as tile
    from contextlib import ExitStack

    dbg_layers = int(os.environ.get("GNN_LAYERS", "3"))
    dbg_bcap = int(os.environ.get("GNN_BATCH_CAP", str(NBATCH)))
    dbg_coll = os.environ.get("GNN_COLL", "1") == "1"
    nqueues = int(os.environ.get("GNN_QUEUES", "4"))
    tagg = os.environ.get("GNN_TAGG", "1") == "1"
    ohb = os.environ.get("GNN_OHB", "0") == "1"
    gchunk = int(os.environ.get("GNN_GCHUNK", "8"))
    scratch = int(os.environ.get("GNN_SCRATCH", "16384"))

    dt = mybir.dt
    nc = bacc.Bacc(
        "TRN2",
        target_bir_lowering=False,
        debug=False,
        num_devices=M,
        num_swdge_queues=nqueues,
        dynamic_dma_scratch_size=scratch,
    )

    t1 = nc.dram_tensor("t1", [TROWS, F], dt.bfloat16, kind="ExternalInput")
    idxd = nc.dram_tensor("idx16", [P, meta["sc16"]], dt.int16, kind="ExternalInput")
    dld = nc.dram_tensor("dl128", [P, meta["sc128"]], dt.bfloat16, kind="ExternalInput")
    disd = nc.dram_tensor("disb", [P, WPD], dt.float32, kind="ExternalInput")
    iod = nc.dram_tensor("iota", [P, 1, P], dt.bfloat16, kind="ExternalInput")
    idnd = nc.dram_tensor("ident", [P, P], dt.bfloat16, kind="ExternalInput")
    w1d = nc.dram_tensor("w1", [F, F], dt.bfloat16, kind="ExternalInput")
    w2d = nc.dram_tensor("w2", [F, F], dt.bfloat16, kind="ExternalInput")
    w3d = nc.dram_tensor("w3", [F, DOUT], dt.bfloat16, kind="ExternalInput")
    b1d = nc.dram_tensor("b1f", [P, F], dt.float32, kind="ExternalInput")
    b2d = nc.dram_tensor("b2f", [P, F], dt.float32, kind="ExternalInput")
    b3d = nc.dram_tensor("b3f", [P, DOUT], dt.float32, kind="ExternalInput")
    outd = nc.dram_tensor("out", [BLKP, DOUT], dt.float32, kind="ExternalOutput")

    with tile.TileContext(nc) as tc, ExitStack() as ctx:
        const = ctx.enter_context(tc.tile_pool(name="const", bufs=1))
        dram = ctx.enter_context(tc.tile_pool(name="dram", bufs=1, space="DRAM"))
        ipool = ctx.enter_context(tc.tile_pool(name="ip", bufs=12))
        dpool = ctx.enter_context(tc.tile_pool(name="dp", bufs=12))
        gpool = ctx.enter_context(tc.tile_pool(name="gp", bufs=6))
        ohpool = ctx.enter_context(tc.tile_pool(name="ohp", bufs=6))
        lhpool = ctx.enter_context(tc.tile_pool(name="lhp", bufs=3))
        zbpool = ctx.enter_context(tc.tile_pool(name="zbp", bufs=4))
        stage = ctx.enter_context(tc.tile_pool(name="stage", bufs=2))
        apsum = ctx.enter_context(tc.tile_pool(name="apsum", bufs=4, space="PSUM"))
        zpsum = ctx.enter_context(tc.tile_pool(name="zpsum", bufs=2, space="PSUM"))
        if not tagg:
            upool = ctx.enter_context(tc.tile_pool(name="up", bufs=3))
            tpsum = ctx.enter_context(tc.tile_pool(name="tpsum", bufs=2, space="PSUM"))

        def cload(name, dram_t, shape, dtype):
            tl = const.tile(shape, dtype, name=name)
            nc.sync.dma_start(out=tl[:], in_=dram_t[:])
            return tl

        iot = cload("iot", iod, [P, 1, P], dt.bfloat16)
        idn = cload("idn", idnd, [P, P], dt.bfloat16)
        dis_t = cload("dis_t", disd, [P, WPD], dt.float32)
        w1t = cload("w1t", w1d, [F, F], dt.bfloat16)
        w2t = cload("w2t", w2d, [F, F], dt.bfloat16)
        w3t = cload("w3t", w3d, [F, DOUT], dt.bfloat16)
        b1t = cload("b1t", b1d, [P, F], dt.float32)
        b2t = cload("b2t", b2d, [P, F], dt.float32)
        b3t = cload("b3t", b3d, [P, DOUT], dt.float32)

        tin2 = dram.tile([BLKP, F], dt.bfloat16, name="tin2")
        tin3 = dram.tile([BLKP, F], dt.bfloat16, name="tin3")
        tf2 = dram.tile([TROWS, F], dt.bfloat16, addr_space="Shared", name="tf2")
        tf3 = dram.tile([TROWS, F], dt.bfloat16, addr_space="Shared", name="tf3")

        calls = meta["calls"]
        gtot = meta["gtot"]

        qctr = [0]

        def do_layer(l, src_of, wt, bt, tst, tin=None, tfull=None):
            gctr = [0] * WPD
            for wb in range(min(NBATCH, dbg_bcap)):
                w0 = wb * WB
                wcnt = min(WB, WPD - w0)
                gts, ohs = [], []
                for p in range(NPAIR):
                    c = calls[(wb, p)]
                    it = ipool.tile([P, c.c16], dt.int16, tag="idx", name=f"it{l}_{wb}_{p}")
                    nc.sync.dma_start(out=it[:], in_=idxd[:, c.ic0 : c.ic0 + c.c16])
                    dt_ = dpool.tile(
                        [P, c.c128, 1], dt.bfloat16, tag="dl", name=f"dl{l}_{wb}_{p}"
                    )
                    nc.sync.dma_start(
                        out=dt_[:],
                        in_=dld[:, c.dc0 : c.dc0 + c.c128].rearrange(
                            "p (c o) -> p c o", o=1
                        ),
                    )
                    gt = gpool.tile(
                        [P, c.c128, F], dt.bfloat16, tag="g", name=f"gt{l}_{wb}_{p}"
                    )
                    for k0 in range(0, c.c128, gchunk):
                        kc = min(gchunk, c.c128 - k0)
                        nc.gpsimd.dma_gather(
                            gt[:, k0 : k0 + kc, :],
                            src_of(p),
                            it[:, k0 * 8 : (k0 + kc) * 8],
                            kc * P,
                            kc * P,
                            F,
                            queue_num=qctr[0] % nqueues,
                        )
                        qctr[0] += 1
                    oh = ohpool.tile(
                        [P, c.c128, P], dt.bfloat16, tag="oh", name=f"oh{l}_{wb}_{p}"
                    )
                    ohstep = c.c128 if ohb else OB
                    for c0 in range(0, c.c128, ohstep):
                        cb = min(ohstep, c.c128 - c0)
                        nc.vector.tensor_tensor(
                            out=oh[:, c0 : c0 + cb, :],
                            in0=dt_[:, c0 : c0 + cb, :].to_broadcast([P, cb, P]),
                            in1=iot[:].to_broadcast([P, cb, P]),
                            op=mybir.AluOpType.is_equal,
                        )
                    gts.append(gt)
                    ohs.append(oh)
                for w in range(w0, w0 + wcnt):
                    # aggregation: tagg => aggT[feat, dst] = sum gt.T @ oh,
                    # else agg[dst, feat] = sum oh.T @ gt
                    agg = apsum.tile([P, P], dt.float32, tag="agg", name=f"agg{l}_{w}")
                    for p in range(NPAIR):
                        c = calls[(wb, p)]
                        glo, ghi = c.winmap[w]
                        for g in range(glo, ghi):
                            st = gctr[w] == 0
                            gctr[w] += 1
                            sp = gctr[w] == gtot[w]
                            nc.tensor.matmul(
                                agg[:],
                                lhsT=gts[p][:, g, :] if tagg else ohs[p][:, g, :],
                                rhs=ohs[p][:, g, :] if tagg else gts[p][:, g, :],
                                start=st,
                                stop=sp,
                            )
                    if tagg:
                        lh = lhpool.tile([P, P], dt.bfloat16, tag="lh", name=f"lh{l}_{w}")
                        nc.vector.tensor_copy(out=lh[:], in_=agg[:])
                    else:
                        u = upool.tile([P, P], dt.bfloat16, tag="u", name=f"u{l}_{w}")
                        nc.vector.tensor_scalar(
                            u[:], agg[:], dis_t[:, w : w + 1], None, mybir.AluOpType.mult
                        )
                        tp = tpsum.tile([P, P], dt.bfloat16, tag="tp", name=f"tp{l}_{w}")
                        nc.tensor.transpose(tp[:], u[:], idn[:])
                        lh = lhpool.tile([P, P], dt.bfloat16, tag="lh", name=f"lh{l}_{w}")
                        nc.vector.tensor_copy(out=lh[:], in_=tp[:])
                    zw = zpsum.tile(
                        [P, F if l < 2 else DOUT], dt.float32, tag="zp", name=f"z{l}_{w}"
                    )
                    nc.tensor.matmul(zw[:], lhsT=lh[:], rhs=wt[:], start=True, stop=True)
                    if tagg:
                        zb = zbpool.tile(
                            [P, F if l < 2 else DOUT], dt.float32, tag="zb",
                            name=f"zb{l}_{w}",
                        )
                        nc.vector.tensor_scalar(
                            zb[:], zw[:], dis_t[:, w : w + 1], None, mybir.AluOpType.mult
                        )
                    else:
                        zb = zw
                    if l < 2:
                        zc = zbpool.tile([P, F], dt.float32, tag="zb", name=f"zc{l}_{w}")
                        nc.vector.tensor_tensor(
                            out=zc[:], in0=zb[:], in1=bt[:], op=mybir.AluOpType.add
                        )
                        nc.scalar.activation(
                            tst[:, w * F : (w + 1) * F],
                            zc[:],
                            mybir.ActivationFunctionType.Relu,
                            scale=dis_t[:, w : w + 1],
                        )
                    else:
                        nc.vector.tensor_tensor(
                            out=tst[:, w * DOUT : (w + 1) * DOUT],
                            in0=zb[:],
                            in1=bt[:],
                            op=mybir.AluOpType.add,
                        )
            if l < 2:
                nc.sync.dma_start(
                    out=tin[:].rearrange("(w p) f -> p w f", p=P),
                    in_=tst[:].rearrange("p (w f) -> p w f", f=F),
                )
                if dbg_coll:
                    nc.gpsimd.collective_compute(
                        "AllGather",
                        mybir.AluOpType.bypass,
                        replica_groups=[list(range(M))],
                        ins=[tin.opt()],
                        outs=[tfull.opt()],
                    )
                else:
                    nc.sync.dma_start(
                        out=tfull[0:BLKP, :].rearrange("(w p) f -> p w f", p=P),
                        in_=tst[:].rearrange("p (w f) -> p w f", f=F),
                    )
            else:
                nc.sync.dma_start(
                    out=outd[:].rearrange("(w p) f -> p w f", p=P),
                    in_=tst[:].rearrange("p (w f) -> p w f", f=DOUT),
                )

        ts1 = stage.tile([P, WPD * F], dt.bfloat16, tag="tstage", name="ts1")
        do_layer(0, lambda p: t1[p * PAIR : (p + 1) * PAIR, :], w1t, b1t, ts1, tin2, tf2)
        if dbg_layers >= 2:
            ts2 = stage.tile([P, WPD * F], dt.bfloat16, tag="tstage", name="ts2")
            do_layer(
                1, lambda p: tf2[p * PAIR : (p + 1) * PAIR, :], w2t, b2t, ts2, tin3, tf3
            )
        if dbg_layers >= 3:
            ts3 = stage.tile([P, WPD * DOUT], dt.float32, tag="tstage", name="ts3")
            do_layer(2, lambda p: tf3[p * PAIR : (p + 1) * PAIR, :], w3t, b3t, ts3)
        else:
            zts = stage.tile([P, WPD * DOUT], dt.float32, tag="tstage", name="zts")
            nc.vector.memset(zts[:], 0.0)
            nc.sync.dma_start(
                out=outd[:].rearrange("(w p) f -> p w f", p=P),
                in_=zts[:].rearrange("p (w f) -> p w f", f=DOUT),
            )

    nc.compile()
    return nc


_CACHE = {}


def _get_program(meta):
    import os

    key = (
        meta["sc16"],
        meta["sc128"],
        os.environ.get("GNN_LAYERS"),
        os.environ.get("GNN_BATCH_CAP"),
        os.environ.get("GNN_COLL"),
        os.environ.get("GNN_QUEUES"),
        os.environ.get("GNN_TAGG"),
        os.environ.get("GNN_OHB"),
        os.environ.get("GNN_GCHUNK"),
        os.environ.get("GNN_SCRATCH"),
    )
    if key not in _CACHE:
        _CACHE[key] = _build_program(meta)
    return _CACHE[key]


def run(trace=False, **inputs):
    from concourse.bass_utils import run_bass_kernel_spmd

    meta, in_maps = _preprocess(**inputs)
    nc = _get_program(meta)
    res = run_bass_kernel_spmd(nc, in_maps, core_ids=list(range(M)), trace=trace)
    out = np.empty((N, DOUT), np.float32)
    for i in range(M):
        out[i * BLK : (i + 1) * BLK] = res.results[i]["out"][:BLK]
    return out, res


def kernel(**inputs):
    out, _ = run(trace=False, **inputs)
    return out



# revision 21
# speedup vs baseline: 1.0459x; 1.0383x over previous
"""3-layer GCN (ContrastiveGNN) on 8 Trainium2 NeuronCores.

Strategy (dst-sharded edge partition, "1D graph partition"):
  - Nodes are split into 8 blocks of 12500 dsts; device i owns block i and all
    edges whose dst lands in its block (plus that block's self-loops).
  - Math reorder: for each GCN layer,
        out = D^-1/2 (A+I) D^-1/2 (h W) + b  ==  dis_d * (sum_{e->d} T[src]) @ W + b
    with T = dis * h (row-scaled activations).  Aggregation happens BEFORE the
    dense transform, so the gather tables carry 128 features for every layer.
  - Aggregation on the tensor engine: edges are sorted by (window-batch,
    src-block-pair, dst-window); each 128-edge group contributes
    one_hot[e, dst_local].T @ gathered[e, feat] accumulated in PSUM per
    128-dst window.  One-hot matrices are built on DVE via iota compare.
  - Gathers use the SWDGE dma_gather custom instruction (int16 indices =>
    the 100352-row table is addressed in 4 block-pair regions of 25088 rows).
  - Tables are bf16 (PSUM accumulation f32); between layers the 8 per-device
    table blocks are exchanged with an AllGather collective.
  - All 8 devices run one SPMD program: per-(batch,pair,window) group counts
    are padded to the max over devices, so instruction streams are identical
    and only the input data (indices, one-hot selectors, dis) differs.
"""

import numpy as np
import ml_dtypes

BF16 = ml_dtypes.bfloat16

N = 100000
F = 128
DOUT = 64
M = 8
BLK = N // M            # 12500 dst nodes per device
P = 128
WPD = (BLK + P - 1) // P  # 98 windows per device
BLKP = WPD * P            # 12544 padded block rows
TROWS = M * BLKP          # 100352 table rows
PAIR = 2 * BLKP           # 25088 rows per src-block-pair region (int16-addressable)
NPAIR = 4
WB = 6                    # windows per gather batch
NBATCH = (WPD + WB - 1) // WB
OB = 8                    # one-hot groups built per DVE op
PADREL = BLKP - 1         # pair-local row of a guaranteed-zero table row


class _Call:
    __slots__ = ("ic0", "c16", "dc0", "c128", "nslots", "wins", "winmap")


def _preprocess(x, edge_index, W1, b1, W2, b2, W3, b3):
    """Host-side index plumbing + input staging. Returns (meta, per-core in_maps,
    reusable static arrays)."""
    x = np.asarray(x, np.float32)
    ei = np.asarray(edge_index)
    src = ei[0].astype(np.int64)
    dst = ei[1].astype(np.int64)
    loop = np.arange(N, dtype=np.int64)
    s_all = np.concatenate([src, loop])
    d_all = np.concatenate([dst, loop])

    deg = np.bincount(d_all, minlength=N).astype(np.float32)
    dis = (1.0 / np.sqrt(deg)).astype(np.float32)

    # layer-1 gather table: dis-scaled input features, block layout with
    # zeroed pad rows (rows BLK..BLKP-1 of each block)
    xs = x * dis[:, None]
    T1 = np.zeros((TROWS, F), BF16)
    for j in range(M):
        T1[j * BLKP : j * BLKP + BLK] = xs[j * BLK : (j + 1) * BLK].astype(BF16)

    dev = d_all // BLK
    j_src = s_all // BLK
    trow = j_src * BLKP + (s_all - j_src * BLK)
    p_pair = j_src // 2
    rel = (trow - p_pair * PAIR).astype(np.int64)  # 0..PAIR-1
    dloc = d_all - dev * BLK
    w_arr = dloc // P
    dwin = dloc - w_arr * P
    wb_arr = w_arr // WB
    bkey = (wb_arr * NPAIR + p_pair) * WPD + w_arr
    NBUCK = NBATCH * NPAIR * WPD

    cnt = np.zeros((M, NBUCK), np.int64)
    for i in range(M):
        cnt[i] = np.bincount(bkey[dev == i], minlength=NBUCK)
    cmax = cnt.max(axis=0)

    # common (SPMD-uniform) padded group counts; >=1 group per valid bucket
    meta_calls = {}
    gtot = np.zeros(WPD, np.int64)
    bucket_order = []   # (bucket_id, slot_offset, padded_slots)
    ic, dc, off = 0, 0, 0
    for wb in range(NBATCH):
        w0 = wb * WB
        wcnt = min(WB, WPD - w0)
        for p in range(NPAIR):
            c = _Call()
            c.ic0, c.dc0 = ic, dc
            c.wins = []
            call_slots = 0
            for w in range(w0, w0 + wcnt):
                bid = (wb * NPAIR + p) * WPD + w
                G = max(1, -(-int(cmax[bid]) // P))
                c.wins.append((w, call_slots // P, call_slots // P + G))
                bucket_order.append((bid, off + call_slots, G * P))
                call_slots += G * P
                gtot[w] += G
            c.nslots = call_slots
            c.c16 = call_slots // 16
            c.c128 = call_slots // P
            c.winmap = {w: (glo, ghi) for (w, glo, ghi) in c.wins}
            ic += c.c16
            dc += c.c128
            off += call_slots
            meta_calls[(wb, p)] = c
    tot_slots = off

    meta = {
        "calls": meta_calls,
        "gtot": gtot,
        "sc16": tot_slots // 16,
        "sc128": tot_slots // P,
        "tot_slots": tot_slots,
    }

    # per-device padded slot arrays
    iota_np = np.tile(np.arange(P, dtype=np.float32).astype(BF16), (P, 1)).reshape(
        P, 1, P
    )
    ident_np = np.eye(P, dtype=np.float32).astype(BF16)
    w1b = np.asarray(W1, np.float32).astype(BF16)
    w2b = np.asarray(W2, np.float32).astype(BF16)
    w3b = np.asarray(W3, np.float32).astype(BF16)
    b1f = np.tile(np.asarray(b1, np.float32), (P, 1))
    b2f = np.tile(np.asarray(b2, np.float32), (P, 1))
    b3f = np.tile(np.asarray(b3, np.float32), (P, 1))

    in_maps = []
    for i in range(M):
        m = dev == i
        bk = bkey[m]
        o = np.argsort(bk, kind="stable")
        bk_s = bk[o]
        rel_s = rel[m][o].astype(np.int16)
        dw_s = dwin[m][o].astype(np.float32)

        idxfl = np.full(tot_slots, PADREL, np.int16)
        dlfl = np.full(tot_slots, -1.0, np.float32)
        bids = np.array([b[0] for b in bucket_order], np.int64)
        starts = np.searchsorted(bk_s, bids)
        for (bid, so, pslots), st in zip(bucket_order, starts):
            cband = int(cnt[i][bid])
            if cband:
                idxfl[so : so + cband] = rel_s[st : st + cband]
                dlfl[so : so + cband] = dw_s[st : st + cband]

        # wrap per call: idx -> [16, c16] tiled to 128 partitions; dl -> [128, c128]
        i16_parts, d128_parts = [], []
        for wb in range(NBATCH):
            for p in range(NPAIR):
                c = meta_calls[(wb, p)]
                so = None
        # offsets per call follow bucket_order grouping; rebuild from cumsum
        off2 = 0
        for wb in range(NBATCH):
            for p in range(NPAIR):
                c = meta_calls[(wb, p)]
                seg_i = idxfl[off2 : off2 + c.nslots]
                seg_d = dlfl[off2 : off2 + c.nslots]
                i16_parts.append(seg_i.reshape(-1, 16).T)
                d128_parts.append(seg_d.reshape(-1, P).T)
                off2 += c.nslots
        idx16 = np.tile(np.concatenate(i16_parts, axis=1), (8, 1))
        dl128 = np.concatenate(d128_parts, axis=1).astype(BF16)

        disp = np.zeros(BLKP, np.float32)
        disp[:BLK] = dis[i * BLK : (i + 1) * BLK]
        disb = disp.reshape(WPD, P).T.copy()

        in_maps.append(
            {
                "t1": T1,
                "idx16": idx16,
                "dl128": dl128,
                "disb": disb,
                "iota": iota_np,
                "ident": ident_np,
                "w1": w1b,
                "w2": w2b,
                "w3": w3b,
                "b1f": b1f,
                "b2f": b2f,
                "b3f": b3f,
            }
        )
    return meta, in_maps


def _build_program(meta):
    import os
    import concourse.bacc as bacc
    import concourse.mybir as mybir
    import concourse.tile as tile
    from contextlib import ExitStack

    dbg_layers = int(os.environ.get("GNN_LAYERS", "3"))
    dbg_bcap = int(os.environ.get("GNN_BATCH_CAP", str(NBATCH)))
    dbg_coll = os.environ.get("GNN_COLL", "1") == "1"
    nqueues = int(os.environ.get("GNN_QUEUES", "4"))
    tagg = os.environ.get("GNN_TAGG", "0") == "1"
    ohb = os.environ.get("GNN_OHB", "0") == "1"
    gchunk = int(os.environ.get("GNN_GCHUNK", "8"))
    scratch = int(os.environ.get("GNN_SCRATCH", "16384"))

    dt = mybir.dt
    nc = bacc.Bacc(
        "TRN2",
        target_bir_lowering=False,
        debug=False,
        num_devices=M,
        num_swdge_queues=nqueues,
        dynamic_dma_scratch_size=scratch,
    )

    t1 = nc.dram_tensor("t1", [TROWS, F], dt.bfloat16, kind="ExternalInput")
    idxd = nc.dram_tensor("idx16", [P, meta["sc16"]], dt.int16, kind="ExternalInput")
    dld = nc.dram_tensor("dl128", [P, meta["sc128"]], dt.bfloat16, kind="ExternalInput")
    disd = nc.dram_tensor("disb", [P, WPD], dt.float32, kind="ExternalInput")
    iod = nc.dram_tensor("iota", [P, 1, P], dt.bfloat16, kind="ExternalInput")
    idnd = nc.dram_tensor("ident", [P, P], dt.bfloat16, kind="ExternalInput")
    w1d = nc.dram_tensor("w1", [F, F], dt.bfloat16, kind="ExternalInput")
    w2d = nc.dram_tensor("w2", [F, F], dt.bfloat16, kind="ExternalInput")
    w3d = nc.dram_tensor("w3", [F, DOUT], dt.bfloat16, kind="ExternalInput")
    b1d = nc.dram_tensor("b1f", [P, F], dt.float32, kind="ExternalInput")
    b2d = nc.dram_tensor("b2f", [P, F], dt.float32, kind="ExternalInput")
    b3d = nc.dram_tensor("b3f", [P, DOUT], dt.float32, kind="ExternalInput")
    outd = nc.dram_tensor("out", [BLKP, DOUT], dt.float32, kind="ExternalOutput")

    with tile.TileContext(nc) as tc, ExitStack() as ctx:
        const = ctx.enter_context(tc.tile_pool(name="const", bufs=1))
        dram = ctx.enter_context(tc.tile_pool(name="dram", bufs=1, space="DRAM"))
        ipool = ctx.enter_context(tc.tile_pool(name="ip", bufs=12))
        dpool = ctx.enter_context(tc.tile_pool(name="dp", bufs=12))
        gpool = ctx.enter_context(tc.tile_pool(name="gp", bufs=6))
        ohpool = ctx.enter_context(tc.tile_pool(name="ohp", bufs=6))
        lhpool = ctx.enter_context(tc.tile_pool(name="lhp", bufs=3))
        zbpool = ctx.enter_context(tc.tile_pool(name="zbp", bufs=4))
        stage = ctx.enter_context(tc.tile_pool(name="stage", bufs=2))
        apsum = ctx.enter_context(tc.tile_pool(name="apsum", bufs=4, space="PSUM"))
        zpsum = ctx.enter_context(tc.tile_pool(name="zpsum", bufs=2, space="PSUM"))
        if not tagg:
            upool = ctx.enter_context(tc.tile_pool(name="up", bufs=3))
            tpsum = ctx.enter_context(tc.tile_pool(name="tpsum", bufs=2, space="PSUM"))

        def cload(name, dram_t, shape, dtype):
            tl = const.tile(shape, dtype, name=name)
            nc.sync.dma_start(out=tl[:], in_=dram_t[:])
            return tl

        iot = cload("iot", iod, [P, 1, P], dt.bfloat16)
        idn = cload("idn", idnd, [P, P], dt.bfloat16)
        dis_t = cload("dis_t", disd, [P, WPD], dt.float32)
        w1t = cload("w1t", w1d, [F, F], dt.bfloat16)
        w2t = cload("w2t", w2d, [F, F], dt.bfloat16)
        w3t = cload("w3t", w3d, [F, DOUT], dt.bfloat16)
        b1t = cload("b1t", b1d, [P, F], dt.float32)
        b2t = cload("b2t", b2d, [P, F], dt.float32)
        b3t = cload("b3t", b3d, [P, DOUT], dt.float32)

        tin2 = dram.tile([BLKP, F], dt.bfloat16, name="tin2")
        tin3 = dram.tile([BLKP, F], dt.bfloat16, name="tin3")
        tf2 = dram.tile([TROWS, F], dt.bfloat16, addr_space="Shared", name="tf2")
        tf3 = dram.tile([TROWS, F], dt.bfloat16, addr_space="Shared", name="tf3")

        calls = meta["calls"]
        gtot = meta["gtot"]

        qctr = [0]

        def do_layer(l, src_of, wt, bt, tst, tin=None, tfull=None):
            gctr = [0] * WPD
            for wb in range(min(NBATCH, dbg_bcap)):
                w0 = wb * WB
                wcnt = min(WB, WPD - w0)
                gts, ohs = [], []
                for p in range(NPAIR):
                    c = calls[(wb, p)]
                    it = ipool.tile([P, c.c16], dt.int16, tag="idx", name=f"it{l}_{wb}_{p}")
                    nc.sync.dma_start(out=it[:], in_=idxd[:, c.ic0 : c.ic0 + c.c16])
                    dt_ = dpool.tile(
                        [P, c.c128, 1], dt.bfloat16, tag="dl", name=f"dl{l}_{wb}_{p}"
                    )
                    nc.sync.dma_start(
                        out=dt_[:],
                        in_=dld[:, c.dc0 : c.dc0 + c.c128].rearrange(
                            "p (c o) -> p c o", o=1
                        ),
                    )
                    gt = gpool.tile(
                        [P, c.c128, F], dt.bfloat16, tag="g", name=f"gt{l}_{wb}_{p}"
                    )
                    for k0 in range(0, c.c128, gchunk):
                        kc = min(gchunk, c.c128 - k0)
                        nc.gpsimd.dma_gather(
                            gt[:, k0 : k0 + kc, :],
                            src_of(p),
                            it[:, k0 * 8 : (k0 + kc) * 8],
                            kc * P,
                            kc * P,
                            F,
                            queue_num=qctr[0] % nqueues,
                        )
                        qctr[0] += 1
                    oh = ohpool.tile(
                        [P, c.c128, P], dt.bfloat16, tag="oh", name=f"oh{l}_{wb}_{p}"
                    )
                    ohstep = c.c128 if ohb else OB
                    for c0 in range(0, c.c128, ohstep):
                        cb = min(ohstep, c.c128 - c0)
                        nc.vector.tensor_tensor(
                            out=oh[:, c0 : c0 + cb, :],
                            in0=dt_[:, c0 : c0 + cb, :].to_broadcast([P, cb, P]),
                            in1=iot[:].to_broadcast([P, cb, P]),
                            op=mybir.AluOpType.is_equal,
                        )
                    gts.append(gt)
                    ohs.append(oh)
                for w in range(w0, w0 + wcnt):
                    # aggregation: tagg => aggT[feat, dst] = sum gt.T @ oh,
                    # else agg[dst, feat] = sum oh.T @ gt
                    agg = apsum.tile([P, P], dt.float32, tag="agg", name=f"agg{l}_{w}")
                    for p in range(NPAIR):
                        c = calls[(wb, p)]
                        glo, ghi = c.winmap[w]
                        for g in range(glo, ghi):
                            st = gctr[w] == 0
                            gctr[w] += 1
                            sp = gctr[w] == gtot[w]
                            nc.tensor.matmul(
                                agg[:],
                                lhsT=gts[p][:, g, :] if tagg else ohs[p][:, g, :],
                                rhs=ohs[p][:, g, :] if tagg else gts[p][:, g, :],
                                start=st,
                                stop=sp,
                            )
                    if tagg:
                        lh = lhpool.tile([P, P], dt.bfloat16, tag="lh", name=f"lh{l}_{w}")
                        nc.vector.tensor_copy(out=lh[:], in_=agg[:])
                    else:
                        u = upool.tile([P, P], dt.bfloat16, tag="u", name=f"u{l}_{w}")
                        nc.vector.tensor_scalar(
                            u[:], agg[:], dis_t[:, w : w + 1], None, mybir.AluOpType.mult
                        )
                        tp = tpsum.tile([P, P], dt.bfloat16, tag="tp", name=f"tp{l}_{w}")
                        nc.tensor.transpose(tp[:], u[:], idn[:])
                        lh = lhpool.tile([P, P], dt.bfloat16, tag="lh", name=f"lh{l}_{w}")
                        nc.vector.tensor_copy(out=lh[:], in_=tp[:])
                    zw = zpsum.tile(
                        [P, F if l < 2 else DOUT], dt.float32, tag="zp", name=f"z{l}_{w}"
                    )
                    nc.tensor.matmul(zw[:], lhsT=lh[:], rhs=wt[:], start=True, stop=True)
                    if tagg:
                        zb = zbpool.tile(
                            [P, F if l < 2 else DOUT], dt.float32, tag="zb",
                            name=f"zb{l}_{w}",
                        )
                        nc.vector.tensor_scalar(
                            zb[:], zw[:], dis_t[:, w : w + 1], None, mybir.AluOpType.mult
                        )
                    else:
                        zb = zw
                    if l < 2:
                        zc = zbpool.tile([P, F], dt.float32, tag="zb", name=f"zc{l}_{w}")
                        nc.vector.tensor_tensor(
                            out=zc[:], in0=zb[:], in1=bt[:], op=mybir.AluOpType.add
                        )
                        nc.scalar.activation(
                            tst[:, w * F : (w + 1) * F],
                            zc[:],
                            mybir.ActivationFunctionType.Relu,
                            scale=dis_t[:, w : w + 1],
                        )
                    else:
                        nc.vector.tensor_tensor(
                            out=tst[:, w * DOUT : (w + 1) * DOUT],
                            in0=zb[:],
                            in1=bt[:],
                            op=mybir.AluOpType.add,
                        )
            if l < 2:
                nc.sync.dma_start(
                    out=tin[:].rearrange("(w p) f -> p w f", p=P),
                    in_=tst[:].rearrange("p (w f) -> p w f", f=F),
                )
                if dbg_coll:
                    nc.gpsimd.collective_compute(
                        "AllGather",
                        mybir.AluOpType.bypass,
                        replica_groups=[list(range(M))],
                        ins=[tin.opt()],
                        outs=[tfull.opt()],
                    )
                else:
                    nc.sync.dma_start(
                        out=tfull[0:BLKP, :].rearrange("(w p) f -> p w f", p=P),
                        in_=tst[:].rearrange("p (w f) -> p w f", f=F),
                    )
            else:
                nc.sync.dma_start(
                    out=outd[:].rearrange("(w p) f -> p w f", p=P),
                    in_=tst[:].rearrange("p (w f) -> p w f", f=DOUT),
                )

        ts1 = stage.tile([P, WPD * F], dt.bfloat16, tag="tstage", name="ts1")
        do_layer(0, lambda p: t1[p * PAIR : (p + 1) * PAIR, :], w1t, b1t, ts1, tin2, tf2)
        if dbg_layers >= 2:
            ts2 = stage.tile([P, WPD * F], dt.bfloat16, tag="tstage", name="ts2")
            do_layer(
                1, lambda p: tf2[p * PAIR : (p + 1) * PAIR, :], w2t, b2t, ts2, tin3, tf3
            )
        if dbg_layers >= 3:
            ts3 = stage.tile([P, WPD * DOUT], dt.float32, tag="tstage", name="ts3")
            do_layer(2, lambda p: tf3[p * PAIR : (p + 1) * PAIR, :], w3t, b3t, ts3)
        else:
            zts = stage.tile([P, WPD * DOUT], dt.float32, tag="tstage", name="zts")
            nc.vector.memset(zts[:], 0.0)
            nc.sync.dma_start(
                out=outd[:].rearrange("(w p) f -> p w f", p=P),
                in_=zts[:].rearrange("p (w f) -> p w f", f=DOUT),
            )

    nc.compile()
    return nc


_CACHE = {}


def _get_program(meta):
    import os

    key = (
        meta["sc16"],
        meta["sc128"],
        os.environ.get("GNN_LAYERS"),
        os.environ.get("GNN_BATCH_CAP"),
        os.environ.get("GNN_COLL"),
        os.environ.get("GNN_QUEUES"),
        os.environ.get("GNN_TAGG"),
        os.environ.get("GNN_OHB"),
        os.environ.get("GNN_GCHUNK"),
        os.environ.get("GNN_SCRATCH"),
    )
    if key not in _CACHE:
        _CACHE[key] = _build_program(meta)
    return _CACHE[key]


def run(trace=False, **inputs):
    from concourse.bass_utils import run_bass_kernel_spmd

    meta, in_maps = _preprocess(**inputs)
    nc = _get_program(meta)
    res = run_bass_kernel_spmd(nc, in_maps, core_ids=list(range(M)), trace=trace)
    out = np.empty((N, DOUT), np.float32)
    for i in range(M):
        out[i * BLK : (i + 1) * BLK] = res.results[i]["out"][:BLK]
    return out, res


def kernel(**inputs):
    out, _ = run(trace=False, **inputs)
    return out



# revision 33
# speedup vs baseline: 1.3254x; 1.2672x over previous
"""3-layer GCN (ContrastiveGNN) on 8 Trainium2 NeuronCores.

Strategy (dst-sharded edge partition, "1D graph partition"):
  - Nodes are split into 8 blocks of 12500 dsts; device i owns block i and all
    edges whose dst lands in its block (plus that block's self-loops).
  - Math reorder: for each GCN layer,
        out = D^-1/2 (A+I) D^-1/2 (h W) + b  ==  dis_d * (sum_{e->d} T[src]) @ W + b
    with T = dis * h (row-scaled activations).  Aggregation happens BEFORE the
    dense transform, so the gather tables carry 128 features for every layer.
  - Aggregation on the tensor engine: edges are sorted by (window-batch,
    src-block-pair, dst-window); each 128-edge group contributes
    one_hot[e, dst_local].T @ gathered[e, feat] accumulated in PSUM per
    128-dst window.  One-hot matrices are built on DVE via iota compare.
  - Gathers use the SWDGE dma_gather custom instruction (int16 indices =>
    the 100352-row table is addressed in 4 block-pair regions of 25088 rows).
  - Tables are bf16 (PSUM accumulation f32); between layers the 8 per-device
    table blocks are exchanged with an AllGather collective.
  - All 8 devices run one SPMD program: per-(batch,pair,window) group counts
    are padded to the max over devices, so instruction streams are identical
    and only the input data (indices, one-hot selectors, dis) differs.
"""

import numpy as np
import ml_dtypes

BF16 = ml_dtypes.bfloat16

N = 100000
F = 128
DOUT = 64
M = 8
BLK = N // M            # 12500 dst nodes per device
P = 128
WPD = (BLK + P - 1) // P  # 98 windows per device
BLKP = WPD * P            # 12544 padded block rows
TROWS = M * BLKP          # 100352 table rows
PAIR = 2 * BLKP           # 25088 rows per src-block-pair region (int16-addressable)
NPAIR = 4
WB = 6                    # windows per gather batch
NBATCH = (WPD + WB - 1) // WB
OB = 8                    # one-hot groups built per DVE op
PADREL = BLKP - 1         # pair-local row of a guaranteed-zero table row

# chunked-AllGather mode: table laid out chunk-major (4 chunks of windows,
# batch-aligned) so each chunk is a contiguous AllGather output AND an
# int16-addressable gather region. Chunk c covers batches CHB[c]..CHB[c+1].
CHB = [0, 5, 9, 13, NBATCH]                     # batch boundaries
CHW = [min((CHB[i + 1]) * WB, WPD) - CHB[i] * WB for i in range(4)]  # windows
CHBASE = [CHB[i] * WB * P for i in range(4)]    # local row base of chunk
CHROWS = [w * P for w in CHW]                   # local rows per chunk
REGB = [M * b for b in CHBASE]                  # table row base of region
REGR = [M * r for r in CHROWS]                  # table rows per region


class _Call:
    __slots__ = ("ic0", "c16", "dc0", "c128", "nslots", "wins", "winmap")


def _preprocess(x, edge_index, W1, b1, W2, b2, W3, b3):
    """Host-side index plumbing + input staging. Returns (meta, per-core in_maps,
    reusable static arrays)."""
    import os

    chunkag = os.environ.get("GNN_CHUNKAG", "0") == "1"
    x = np.asarray(x, np.float32)
    ei = np.asarray(edge_index)
    src = ei[0].astype(np.int64)
    dst = ei[1].astype(np.int64)
    loop = np.arange(N, dtype=np.int64)
    s_all = np.concatenate([src, loop])
    d_all = np.concatenate([dst, loop])

    deg = np.bincount(d_all, minlength=N).astype(np.float32)
    dis = (1.0 / np.sqrt(deg)).astype(np.float32)

    # layer-1 gather table: dis-scaled input features, zeroed pad rows
    xs = x * dis[:, None]
    T1 = np.zeros((TROWS, F), BF16)
    if chunkag:
        # chunk-major layout: region c = [M blocks x CHROWS[c] local rows]
        for c in range(4):
            for j in range(M):
                lo = j * BLK + CHBASE[c]
                hi = min(j * BLK + CHBASE[c] + CHROWS[c], (j + 1) * BLK)
                db = REGB[c] + j * CHROWS[c]
                T1[db : db + (hi - lo)] = xs[lo:hi].astype(BF16)
    else:
        for j in range(M):
            T1[j * BLKP : j * BLKP + BLK] = xs[j * BLK : (j + 1) * BLK].astype(BF16)

    dev = d_all // BLK
    j_src = s_all // BLK
    loc_src = s_all - j_src * BLK
    if chunkag:
        # src chunk id + region-local row
        c_src = np.minimum(loc_src // 3840, 1) + (loc_src >= 6912) + (
            loc_src >= 9984
        )
        chrows = np.array(CHROWS, np.int64)
        chbase = np.array(CHBASE, np.int64)
        p_pair = c_src
        rel = (j_src * chrows[c_src] + (loc_src - chbase[c_src])).astype(np.int64)
        padrel = 0
    else:
        trow = j_src * BLKP + loc_src
        p_pair = j_src // 2
        rel = (trow - p_pair * PAIR).astype(np.int64)  # 0..PAIR-1
        padrel = PADREL
    dloc = d_all - dev * BLK
    w_arr = dloc // P
    dwin = dloc - w_arr * P
    wb_arr = w_arr // WB
    bkey = (wb_arr * NPAIR + p_pair) * WPD + w_arr
    NBUCK = NBATCH * NPAIR * WPD

    cnt = np.zeros((M, NBUCK), np.int64)
    for i in range(M):
        cnt[i] = np.bincount(bkey[dev == i], minlength=NBUCK)
    cmax = cnt.max(axis=0)

    # common (SPMD-uniform) padded group counts; >=1 group per valid bucket
    meta_calls = {}
    gtot = np.zeros(WPD, np.int64)
    bucket_order = []   # (bucket_id, slot_offset, padded_slots)
    ic, dc, off = 0, 0, 0
    for wb in range(NBATCH):
        w0 = wb * WB
        wcnt = min(WB, WPD - w0)
        for p in range(NPAIR):
            c = _Call()
            c.ic0, c.dc0 = ic, dc
            c.wins = []
            call_slots = 0
            for w in range(w0, w0 + wcnt):
                bid = (wb * NPAIR + p) * WPD + w
                G = max(1, -(-int(cmax[bid]) // P))
                c.wins.append((w, call_slots // P, call_slots // P + G))
                bucket_order.append((bid, off + call_slots, G * P))
                call_slots += G * P
                gtot[w] += G
            c.nslots = call_slots
            c.c16 = call_slots // 16
            c.c128 = call_slots // P
            c.winmap = {w: (glo, ghi) for (w, glo, ghi) in c.wins}
            ic += c.c16
            dc += c.c128
            off += call_slots
            meta_calls[(wb, p)] = c
    tot_slots = off

    meta = {
        "calls": meta_calls,
        "gtot": gtot,
        "sc16": tot_slots // 16,
        "sc128": tot_slots // P,
        "tot_slots": tot_slots,
    }

    # per-device padded slot arrays
    iota_np = np.tile(np.arange(P, dtype=np.float32).astype(BF16), (P, 1)).reshape(
        P, 1, P
    )
    ident_np = np.eye(P, dtype=np.float32).astype(BF16)
    w1b = np.asarray(W1, np.float32).astype(BF16)
    w2b = np.asarray(W2, np.float32).astype(BF16)
    w3b = np.asarray(W3, np.float32).astype(BF16)
    b1f = np.tile(np.asarray(b1, np.float32), (P, 1))
    b2f = np.tile(np.asarray(b2, np.float32), (P, 1))
    b3f = np.tile(np.asarray(b3, np.float32), (P, 1))

    in_maps = []
    for i in range(M):
        m = dev == i
        bk = bkey[m]
        o = np.argsort(bk, kind="stable")
        bk_s = bk[o]
        rel_s = rel[m][o].astype(np.int16)
        dw_s = dwin[m][o].astype(np.float32)

        idxfl = np.full(tot_slots, padrel, np.int16)
        dlfl = np.full(tot_slots, -1.0, np.float32)
        bids = np.array([b[0] for b in bucket_order], np.int64)
        starts = np.searchsorted(bk_s, bids)
        for (bid, so, pslots), st in zip(bucket_order, starts):
            cband = int(cnt[i][bid])
            if cband:
                idxfl[so : so + cband] = rel_s[st : st + cband]
                dlfl[so : so + cband] = dw_s[st : st + cband]

        # wrap per call: idx -> [16, c16] tiled to 128 partitions; dl -> [128, c128]
        i16_parts, d128_parts = [], []
        for wb in range(NBATCH):
            for p in range(NPAIR):
                c = meta_calls[(wb, p)]
                so = None
        # offsets per call follow bucket_order grouping; rebuild from cumsum
        off2 = 0
        for wb in range(NBATCH):
            for p in range(NPAIR):
                c = meta_calls[(wb, p)]
                seg_i = idxfl[off2 : off2 + c.nslots]
                seg_d = dlfl[off2 : off2 + c.nslots]
                i16_parts.append(seg_i.reshape(-1, 16).T)
                d128_parts.append(seg_d.reshape(-1, P).T)
                off2 += c.nslots
        idx16 = np.tile(np.concatenate(i16_parts, axis=1), (8, 1))
        dl128 = np.concatenate(d128_parts, axis=1).astype(BF16)

        disp = np.zeros(BLKP, np.float32)
        disp[:BLK] = dis[i * BLK : (i + 1) * BLK]
        disb = disp.reshape(WPD, P).T.copy()

        in_maps.append(
            {
                "t1": T1,
                "idx16": idx16,
                "dl128": dl128,
                "disb": disb,
                "iota": iota_np,
                "ident": ident_np,
                "w1": w1b,
                "w2": w2b,
                "w3": w3b,
                "b1f": b1f,
                "b2f": b2f,
                "b3f": b3f,
            }
        )
    return meta, in_maps


def _build_program(meta):
    import os
    import concourse.bacc as bacc
    import concourse.mybir as mybir
    import concourse.tile as tile
    from contextlib import ExitStack

    dbg_layers = int(os.environ.get("GNN_LAYERS", "3"))
    dbg_bcap = int(os.environ.get("GNN_BATCH_CAP", str(NBATCH)))
    dbg_coll = os.environ.get("GNN_COLL", "1") == "1"
    nqueues = int(os.environ.get("GNN_QUEUES", "4"))
    tagg = os.environ.get("GNN_TAGG", "0") == "1"
    ohb = os.environ.get("GNN_OHB", "0") == "1"
    gchunk = int(os.environ.get("GNN_GCHUNK", "8"))
    scratch = int(os.environ.get("GNN_SCRATCH", "16384"))
    chunkag = os.environ.get("GNN_CHUNKAG", "0") == "1"

    dt = mybir.dt
    nc = bacc.Bacc(
        "TRN2",
        target_bir_lowering=False,
        debug=False,
        num_devices=M,
        num_swdge_queues=nqueues,
        dynamic_dma_scratch_size=scratch,
    )

    t1 = nc.dram_tensor("t1", [TROWS, F], dt.bfloat16, kind="ExternalInput")
    idxd = nc.dram_tensor("idx16", [P, meta["sc16"]], dt.int16, kind="ExternalInput")
    dld = nc.dram_tensor("dl128", [P, meta["sc128"]], dt.bfloat16, kind="ExternalInput")
    disd = nc.dram_tensor("disb", [P, WPD], dt.float32, kind="ExternalInput")
    iod = nc.dram_tensor("iota", [P, 1, P], dt.bfloat16, kind="ExternalInput")
    idnd = nc.dram_tensor("ident", [P, P], dt.bfloat16, kind="ExternalInput")
    w1d = nc.dram_tensor("w1", [F, F], dt.bfloat16, kind="ExternalInput")
    w2d = nc.dram_tensor("w2", [F, F], dt.bfloat16, kind="ExternalInput")
    w3d = nc.dram_tensor("w3", [F, DOUT], dt.bfloat16, kind="ExternalInput")
    b1d = nc.dram_tensor("b1f", [P, F], dt.float32, kind="ExternalInput")
    b2d = nc.dram_tensor("b2f", [P, F], dt.float32, kind="ExternalInput")
    b3d = nc.dram_tensor("b3f", [P, DOUT], dt.float32, kind="ExternalInput")
    outd = nc.dram_tensor("out", [BLKP, DOUT], dt.float32, kind="ExternalOutput")

    with tile.TileContext(nc) as tc, ExitStack() as ctx:
        const = ctx.enter_context(tc.tile_pool(name="const", bufs=1))
        dram = ctx.enter_context(tc.tile_pool(name="dram", bufs=1, space="DRAM"))
        ipool = ctx.enter_context(tc.tile_pool(name="ip", bufs=12))
        dpool = ctx.enter_context(tc.tile_pool(name="dp", bufs=12))
        gpool = ctx.enter_context(tc.tile_pool(name="gp", bufs=6))
        ohpool = ctx.enter_context(tc.tile_pool(name="ohp", bufs=6))
        lhpool = ctx.enter_context(tc.tile_pool(name="lhp", bufs=3))
        zbpool = ctx.enter_context(tc.tile_pool(name="zbp", bufs=4))
        stage = ctx.enter_context(tc.tile_pool(name="stage", bufs=2))
        apsum = ctx.enter_context(tc.tile_pool(name="apsum", bufs=4, space="PSUM"))
        zpsum = ctx.enter_context(tc.tile_pool(name="zpsum", bufs=2, space="PSUM"))
        if not tagg:
            upool = ctx.enter_context(tc.tile_pool(name="up", bufs=3))
            tpsum = ctx.enter_context(tc.tile_pool(name="tpsum", bufs=2, space="PSUM"))

        def cload(name, dram_t, shape, dtype):
            tl = const.tile(shape, dtype, name=name)
            nc.sync.dma_start(out=tl[:], in_=dram_t[:])
            return tl

        iot = cload("iot", iod, [P, 1, P], dt.bfloat16)
        idn = cload("idn", idnd, [P, P], dt.bfloat16)
        dis_t = cload("dis_t", disd, [P, WPD], dt.float32)
        w1t = cload("w1t", w1d, [F, F], dt.bfloat16)
        w2t = cload("w2t", w2d, [F, F], dt.bfloat16)
        w3t = cload("w3t", w3d, [F, DOUT], dt.bfloat16)
        b1t = cload("b1t", b1d, [P, F], dt.float32)
        b2t = cload("b2t", b2d, [P, F], dt.float32)
        b3t = cload("b3t", b3d, [P, DOUT], dt.float32)

        tin2 = dram.tile([BLKP, F], dt.bfloat16, name="tin2")
        tin3 = dram.tile([BLKP, F], dt.bfloat16, name="tin3")
        if chunkag:
            tf2 = [
                dram.tile([REGR[c], F], dt.bfloat16, addr_space="Shared", name=f"tf2_{c}")
                for c in range(4)
            ]
            tf3 = [
                dram.tile([REGR[c], F], dt.bfloat16, addr_space="Shared", name=f"tf3_{c}")
                for c in range(4)
            ]
        else:
            tf2 = dram.tile([TROWS, F], dt.bfloat16, addr_space="Shared", name="tf2")
            tf3 = dram.tile([TROWS, F], dt.bfloat16, addr_space="Shared", name="tf3")

        calls = meta["calls"]
        gtot = meta["gtot"]

        qctr = [0]

        def stage_chunk(l, c, tst, tin, tfull):
            """Write stage chunk c to tin and AllGather it into tfull."""
            fw = F if l < 2 else DOUT
            r0, rn, w0, wn = CHBASE[c], CHROWS[c], CHB[c] * WB, CHW[c]
            nc.sync.dma_start(
                out=tin[r0 : r0 + rn, :].rearrange("(w p) f -> p w f", p=P),
                in_=tst[:, w0 * fw : (w0 + wn) * fw].rearrange(
                    "p (w f) -> p w f", f=fw
                ),
            )
            if dbg_coll:
                nc.gpsimd.collective_compute(
                    "AllGather",
                    mybir.AluOpType.bypass,
                    replica_groups=[list(range(M))],
                    ins=[tin[r0 : r0 + rn, :].opt()],
                    outs=[tfull[c].opt()],
                )
            else:
                nc.sync.dma_start(
                    out=tfull[c][0:rn, :].rearrange("(w p) f -> p w f", p=P),
                    in_=tst[:, w0 * fw : (w0 + wn) * fw].rearrange(
                        "p (w f) -> p w f", f=fw
                    ),
                )

        def do_layer(l, src_of, wt, bt, tst, tin=None, tfull=None):
            gctr = [0] * WPD
            for wb in range(min(NBATCH, dbg_bcap)):
                w0 = wb * WB
                wcnt = min(WB, WPD - w0)
                gts, ohs = [], []
                for p in range(NPAIR):
                    c = calls[(wb, p)]
                    it = ipool.tile([P, c.c16], dt.int16, tag="idx", name=f"it{l}_{wb}_{p}")
                    nc.sync.dma_start(out=it[:], in_=idxd[:, c.ic0 : c.ic0 + c.c16])
                    dt_ = dpool.tile(
                        [P, c.c128, 1], dt.bfloat16, tag="dl", name=f"dl{l}_{wb}_{p}"
                    )
                    nc.sync.dma_start(
                        out=dt_[:],
                        in_=dld[:, c.dc0 : c.dc0 + c.c128].rearrange(
                            "p (c o) -> p c o", o=1
                        ),
                    )
                    gt = gpool.tile(
                        [P, c.c128, F], dt.bfloat16, tag="g", name=f"gt{l}_{wb}_{p}"
                    )
                    for k0 in range(0, c.c128, gchunk):
                        kc = min(gchunk, c.c128 - k0)
                        nc.gpsimd.dma_gather(
                            gt[:, k0 : k0 + kc, :],
                            src_of(p),
                            it[:, k0 * 8 : (k0 + kc) * 8],
                            kc * P,
                            kc * P,
                            F,
                            queue_num=qctr[0] % nqueues,
                        )
                        qctr[0] += 1
                    oh = ohpool.tile(
                        [P, c.c128, P], dt.bfloat16, tag="oh", name=f"oh{l}_{wb}_{p}"
                    )
                    ohstep = c.c128 if ohb else OB
                    for c0 in range(0, c.c128, ohstep):
                        cb = min(ohstep, c.c128 - c0)
                        nc.vector.tensor_tensor(
                            out=oh[:, c0 : c0 + cb, :],
                            in0=dt_[:, c0 : c0 + cb, :].to_broadcast([P, cb, P]),
                            in1=iot[:].to_broadcast([P, cb, P]),
                            op=mybir.AluOpType.is_equal,
                        )
                    gts.append(gt)
                    ohs.append(oh)
                for w in range(w0, w0 + wcnt):
                    # aggregation: tagg => aggT[feat, dst] = sum gt.T @ oh,
                    # else agg[dst, feat] = sum oh.T @ gt
                    agg = apsum.tile([P, P], dt.float32, tag="agg", name=f"agg{l}_{w}")
                    for p in range(NPAIR):
                        c = calls[(wb, p)]
                        glo, ghi = c.winmap[w]
                        for g in range(glo, ghi):
                            st = gctr[w] == 0
                            gctr[w] += 1
                            sp = gctr[w] == gtot[w]
                            nc.tensor.matmul(
                                agg[:],
                                lhsT=gts[p][:, g, :] if tagg else ohs[p][:, g, :],
                                rhs=ohs[p][:, g, :] if tagg else gts[p][:, g, :],
                                start=st,
                                stop=sp,
                            )
                    if tagg:
                        lh = lhpool.tile([P, P], dt.bfloat16, tag="lh", name=f"lh{l}_{w}")
                        nc.vector.tensor_copy(out=lh[:], in_=agg[:])
                    else:
                        u = upool.tile([P, P], dt.bfloat16, tag="u", name=f"u{l}_{w}")
                        nc.vector.tensor_scalar(
                            u[:], agg[:], dis_t[:, w : w + 1], None, mybir.AluOpType.mult
                        )
                        tp = tpsum.tile([P, P], dt.bfloat16, tag="tp", name=f"tp{l}_{w}")
                        nc.tensor.transpose(tp[:], u[:], idn[:])
                        lh = lhpool.tile([P, P], dt.bfloat16, tag="lh", name=f"lh{l}_{w}")
                        nc.vector.tensor_copy(out=lh[:], in_=tp[:])
                    zw = zpsum.tile(
                        [P, F if l < 2 else DOUT], dt.float32, tag="zp", name=f"z{l}_{w}"
                    )
                    nc.tensor.matmul(zw[:], lhsT=lh[:], rhs=wt[:], start=True, stop=True)
                    if tagg:
                        zb = zbpool.tile(
                            [P, F if l < 2 else DOUT], dt.float32, tag="zb",
                            name=f"zb{l}_{w}",
                        )
                        nc.vector.tensor_scalar(
                            zb[:], zw[:], dis_t[:, w : w + 1], None, mybir.AluOpType.mult
                        )
                    else:
                        zb = zw
                    if l < 2:
                        zc = zbpool.tile([P, F], dt.float32, tag="zb", name=f"zc{l}_{w}")
                        nc.vector.tensor_tensor(
                            out=zc[:], in0=zb[:], in1=bt[:], op=mybir.AluOpType.add
                        )
                        nc.scalar.activation(
                            tst[:, w * F : (w + 1) * F],
                            zc[:],
                            mybir.ActivationFunctionType.Relu,
                            scale=dis_t[:, w : w + 1],
                        )
                    else:
                        nc.vector.tensor_tensor(
                            out=tst[:, w * DOUT : (w + 1) * DOUT],
                            in0=zb[:],
                            in1=bt[:],
                            op=mybir.AluOpType.add,
                        )
                if chunkag and l < 2 and (wb + 1) in CHB[1:]:
                    stage_chunk(l, CHB.index(wb + 1) - 1, tst, tin, tfull)
            if l < 2:
                if chunkag:
                    pass  # chunks already staged + gathered in-loop
                else:
                    nc.sync.dma_start(
                        out=tin[:].rearrange("(w p) f -> p w f", p=P),
                        in_=tst[:].rearrange("p (w f) -> p w f", f=F),
                    )
                    if dbg_coll:
                        nc.gpsimd.collective_compute(
                            "AllGather",
                            mybir.AluOpType.bypass,
                            replica_groups=[list(range(M))],
                            ins=[tin.opt()],
                            outs=[tfull.opt()],
                        )
                    else:
                        nc.sync.dma_start(
                            out=tfull[0:BLKP, :].rearrange("(w p) f -> p w f", p=P),
                            in_=tst[:].rearrange("p (w f) -> p w f", f=F),
                        )
            else:
                nc.sync.dma_start(
                    out=outd[:].rearrange("(w p) f -> p w f", p=P),
                    in_=tst[:].rearrange("p (w f) -> p w f", f=DOUT),
                )

        if chunkag:
            reg = lambda t: (
                (lambda p: t[REGB[p] : REGB[p] + REGR[p], :])
                if not isinstance(t, list)
                else (lambda p: t[p][:])
            )
        else:
            reg = lambda t: (lambda p: t[p * PAIR : (p + 1) * PAIR, :])

        ts1 = stage.tile([P, WPD * F], dt.bfloat16, tag="tstage", name="ts1")
        do_layer(0, reg(t1), w1t, b1t, ts1, tin2, tf2)
        if dbg_layers >= 2:
            ts2 = stage.tile([P, WPD * F], dt.bfloat16, tag="tstage", name="ts2")
            do_layer(1, reg(tf2), w2t, b2t, ts2, tin3, tf3)
        if dbg_layers >= 3:
            ts3 = stage.tile([P, WPD * DOUT], dt.float32, tag="tstage", name="ts3")
            do_layer(2, reg(tf3), w3t, b3t, ts3)
        else:
            zts = stage.tile([P, WPD * DOUT], dt.float32, tag="tstage", name="zts")
            nc.vector.memset(zts[:], 0.0)
            nc.sync.dma_start(
                out=outd[:].rearrange("(w p) f -> p w f", p=P),
                in_=zts[:].rearrange("p (w f) -> p w f", f=DOUT),
            )

    nc.compile()
    return nc


_CACHE = {}


def _get_program(meta):
    import os

    key = (
        meta["sc16"],
        meta["sc128"],
        os.environ.get("GNN_LAYERS"),
        os.environ.get("GNN_BATCH_CAP"),
        os.environ.get("GNN_COLL"),
        os.environ.get("GNN_QUEUES"),
        os.environ.get("GNN_TAGG"),
        os.environ.get("GNN_OHB"),
        os.environ.get("GNN_GCHUNK"),
        os.environ.get("GNN_SCRATCH"),
        os.environ.get("GNN_CHUNKAG"),
    )
    if key not in _CACHE:
        _CACHE[key] = _build_program(meta)
    return _CACHE[key]


def run(trace=False, **inputs):
    from concourse.bass_utils import run_bass_kernel_spmd

    meta, in_maps = _preprocess(**inputs)
    nc = _get_program(meta)
    res = run_bass_kernel_spmd(nc, in_maps, core_ids=list(range(M)), trace=trace)
    out = np.empty((N, DOUT), np.float32)
    for i in range(M):
        out[i * BLK : (i + 1) * BLK] = res.results[i]["out"][:BLK]
    return out, res


def kernel(**inputs):
    out, _ = run(trace=False, **inputs)
    return out

